# revision 46
# baseline (speedup 1.0000x reference)
"""Trainium2 Bass kernel for nn_AttentionBlock_33724083208839 (sparse_attention).

Data-parallel over batch (8 batches -> 8 cores). Per core:
  1. load x (chunked), transpose via PE -> xT f32; K projected in f32
     (exact, feeds selection); Q/V projected in bf16; Q also written
     row-major to DRAM (padded to 128 cols) for the later gather.
  2. K_reduce via the exact CVaR identity sum_top_l = l*t + sum(relu(x-t));
     sqk = x @ (Wq @ K_reduce) f32 on PE; exact top-l membership via a
     5-pass 128-ary threshold search on a partition-replicated sqk.
  3. mask -> compacted query-index list on GPSIMD (sparse_gather), tail
     padded with -1; selected Q columns gathered transposed from DRAM
     via dma_gather(transpose) in one shot.
  4. attention ONLY for the 2744 selected queries (2816 padded columns):
     bf16 scores -> exp split between ACT (native) and DVE (Schraudolph
     bit-trick through an int16 view) -> [V|1]^T @ P^T on PE -> transpose,
     normalize.  Output = meanV rows (masked fill, DMA'd early) plus a
     dma_scatter_add of the normalized attention rows at the selected
     indices (selected base rows are zeroed so add == set).
"""
import sys

sys.path.insert(0, "/opt/trn_rl_repo")

import math
from statistics import NormalDist

import numpy as np

import concourse.bacc as bacc
import concourse.bass as bass
import concourse.bass_isa as bass_isa
import concourse.mybir as mybir
from concourse.tile import TileContext
from concourse.masks import make_identity
from concourse.bass_utils import run_bass_kernel_spmd

B, L, D = 8, 4096, 64
LQ = int((1.0 - 0.33) * L)  # 2744
PART = 128
NT = L // PART
N_CORES = 8

NSEL = 2816                  # padded selected-column count (22*128, 5.5 slabs)
NTS = NSEL // PART           # 22 column-tiles of selected queries
SLABS = [512, 512, 512, 512, 512, 256]
assert sum(SLABS) == NSEL

QFRAC = 1.0 - LQ / L
Z = NormalDist().inv_cdf(QFRAC)
PHI = math.exp(-Z * Z / 2.0) / math.sqrt(2.0 * math.pi)

f32 = mybir.dt.float32
bf16 = mybir.dt.bfloat16
u8 = mybir.dt.uint8
u32 = mybir.dt.uint32
i16 = mybir.dt.int16
i32 = mybir.dt.int32
AF = mybir.ActivationFunctionType
OP = mybir.AluOpType

N_PASS = 4          # 65-ary search on 64 replicated partitions
NCAND = 64
BOUND = 64.0        # |sqk| stays well inside (values above just count high)

# Schraudolph exp into bf16 bits (through an i16 view): i = A*s + B, with
# the 0.125 score scale folded into A; B centered for truncating convert.
A_S = (2.0 ** 7 / math.log(2.0)) * 0.125
B_S = 127.0 * 2 ** 7 - 6.873

# 2-tile score groups (one PSUM strip each); exp engine per group.
GROUPS = [(g, min(2, NT - g)) for g in range(0, NT, 2)]
EXP_ENG = ['act', 'act', 'dve', 'act', 'act', 'act', 'dve', 'act',
           'act', 'act', 'act', 'act', 'act', 'dve', 'act', 'act']


def build(debug: bool = False):
    nc = bacc.Bacc("TRN2")
    x = nc.dram_tensor("x", [L, D], f32, kind="ExternalInput")
    wq = nc.dram_tensor("Wq", [D, D], f32, kind="ExternalInput")
    wk = nc.dram_tensor("Wk", [D, D], f32, kind="ExternalInput")
    wv = nc.dram_tensor("Wv", [D, D], f32, kind="ExternalInput")
    out = nc.dram_tensor("out", [L, D], f32, kind="ExternalOutput")
    scr_row = nc.dram_tensor("scr_row", [1, L], f32, kind="Internal")
    scr_sel = nc.dram_tensor("scr_sel", [PART, NT], f32, kind="Internal")
    scr_idx = nc.dram_tensor("scr_idx", [1, 16 * 2 * (NSEL // 16)], i16, kind="Internal")
    q_rows = nc.dram_tensor("q_rows", [L, PART], bf16, kind="Internal")
    dbg = {}
    if debug:
        for name, shape in [
            ("dbg_kr", [D, 1]), ("dbg_thr", [PART, 1]), ("dbg_sqk", [PART, NT]),
            ("dbg_mask", [PART, NT]), ("dbg_idx", [16, NSEL // 16]),
            ("dbg_cnt", [1, 1]),
        ]:
            dt = f32 if name != "dbg_cnt" else f32
            dbg[name] = nc.dram_tensor(name, shape, dt, kind="ExternalOutput")

    x_re = x[:].rearrange("(c p) d -> p c d", p=PART)
    out_re = out[:].rearrange("(c p) d -> p c d", p=PART)
    q_rows_re = q_rows[:].rearrange("(c p) d -> p c d", p=PART)

    with TileContext(nc) as tc, \
         tc.tile_pool(name="cst", bufs=1) as cst, \
         tc.tile_pool(name="big", bufs=1) as big, \
         tc.tile_pool(name="sc", bufs=2) as sc, \
         tc.tile_pool(name="mn", bufs=2) as mn:

        # ---- warm the exp activation table immediately ----
        warm = cst.tile([1, 8], f32)
        nc.vector.memset(warm[:], 0.0)
        warm2 = cst.tile([1, 8], f32)
        nc.scalar.activation(out=warm2[:], in_=warm[:], func=AF.Exp)

        # ---- constants ----
        ident = cst.tile([PART, PART], f32)
        make_identity(nc, ident[:])
        onesb = cst.tile([PART, 1], bf16)
        nc.vector.memset(onesb[:], 1.0)
        ones1x128 = cst.tile([1, PART], f32)
        nc.vector.memset(ones1x128[:], 1.0)
        pidx1i = cst.tile([PART, 1], i32)
        nc.gpsimd.iota(pidx1i[:], pattern=[[1, 1]], base=1, channel_multiplier=1)
        pidx1 = cst.tile([PART, 1], f32)
        nc.vector.tensor_copy(pidx1[:], pidx1i[:])
        # query index + 1 per (partition, tile): q = c*128 + p
        qio_i = cst.tile([PART, NT], i32)
        nc.gpsimd.iota(qio_i[:], pattern=[[PART, NT]], base=1, channel_multiplier=1)
        qio = cst.tile([PART, NT], f32)
        nc.vector.tensor_copy(qio[:], qio_i[:])
        # wrap16 helpers
        rio_i = cst.tile([16, 1], i32)
        nc.gpsimd.iota(rio_i[:], pattern=[[1, 1]], base=0, channel_multiplier=1)
        rio = cst.tile([16, 1], f32)
        nc.vector.tensor_copy(rio[:], rio_i[:])
        fio_i = cst.tile([16, NSEL // 16], i32)
        nc.gpsimd.iota(fio_i[:], pattern=[[1, NSEL // 16]], base=0, channel_multiplier=0)
        fio = cst.tile([16, NSEL // 16], f32)
        nc.vector.tensor_copy(fio[:], fio_i[:])

        # ---- persistent tensors ----
        xT32 = big.tile([D, L], f32)
        xT16 = big.tile([D, L], bf16)
        kT32 = big.tile([D, L], f32)
        kT = big.tile([D, L], bf16)
        qselT = big.tile([PART, NSEL], bf16)
        vp = big.tile([PART, NT, D + 1], bf16)
        pt_a = big.tile([PART, NT, 512], bf16)
        pt_b = big.tile([PART, NT, 512], bf16)
        res_sel = big.tile([PART, NTS, D], f32)
        mvf = big.tile([PART, D], f32)
        mask = big.tile([PART, NT], f32)
        sqk = big.tile([PART, NT], f32)
        kr = big.tile([D, 1], f32)
        wvec = big.tile([D, 1], f32)
        sqk_rep = big.tile([NCAND, L], f32)
        cmp_rep = big.tile([NCAND, L], f32)
        tmp1m = cmp_rep[0:D, :]  # scratch view; K_reduce precedes the search
        idxbo = big.tile([16, 2, NSEL // 16], i16)
        idx2 = big.tile([PART, 2, NSEL // 16], i16)   # [:,0]=-1-padded, [:,1]=0-clamped

        # weights
        wq_s = cst.tile([D, D], f32)
        wk_s = cst.tile([D, D], f32)
        wv_s = cst.tile([D, D], f32)
        nc.sync.dma_start(out=wq_s[:], in_=wq[:])
        nc.sync.dma_start(out=wk_s[:], in_=wk[:])
        nc.sync.dma_start(out=wv_s[:], in_=wv[:])
        wq16 = cst.tile([D, D], bf16)
        nc.vector.tensor_copy(wq16[:], wq_s[:])
        wv16 = cst.tile([D, D], bf16)
        nc.vector.tensor_copy(wv16[:], wv_s[:])

        def emit_exp(gi, strip_ap, ptc, g0, glen, w):
            eng = EXP_ENG[gi]
            dst = ptc[:, g0:g0 + glen, 0:w]
            if eng == 'act':
                nc.scalar.activation(out=dst, in_=strip_ap, func=AF.Exp,
                                     scale=0.125)
            else:
                nc.vector.tensor_scalar(out=dst.bitcast(i16), in0=strip_ap,
                                        scalar1=float(A_S), scalar2=float(B_S),
                                        op0=OP.mult, op1=OP.add)

        # =============== phase 1: load / transpose / project ===============
        with tc.tile_pool(name="ps_xv", bufs=2, space="PSUM") as ps_xv, \
             tc.tile_pool(name="ps_pj", bufs=2, space="PSUM") as ps_pj, \
             tc.tile_pool(name="ps_qr", bufs=2, space="PSUM") as ps_qr, \
             tc.tile_pool(name="xl", bufs=3) as xl:

            def load_tiles(c0, c1):
                xt = xl.tile([PART, 4, D], f32, tag="x_ld")
                nc.sync.dma_start(out=xt[:, 0:c1 - c0, :], in_=x_re[:, c0:c1, :])
                for c in range(c0, c1):
                    pxt = ps_xv.tile([PART, PART], f32, tag="xv")
                    nc.tensor.transpose(out=pxt[0:D, :], in_=xt[:, c - c0, :],
                                        identity=ident[:])
                    nc.scalar.copy(xT32[:, PART * c:PART * (c + 1)], pxt[0:D, :])

            def proj_slab(s):
                sl = slice(512 * s, 512 * (s + 1))
                nc.gpsimd.tensor_copy(xT16[:, sl], xT32[:, sl])
                pk = ps_pj.tile([D, 512], f32, tag="pj")
                nc.tensor.matmul(out=pk[:], lhsT=wk_s[:], rhs=xT32[:, sl],
                                 start=True, stop=True)
                nc.vector.tensor_copy(kT32[:, sl], pk[:])
                nc.gpsimd.tensor_copy(kT[:, sl], kT32[:, sl])
                # K_reduce statistics per slab (DVE)
                nc.vector.bn_stats(bstats[:, s, :], kT32[:, sl])

            def proj_qv(c0, c1):
                for c in range(c0, c1):
                    # V row-tile [128, 64]
                    pv = ps_xv.tile([PART, PART], f32, tag="xv")
                    nc.tensor.matmul(out=pv[:, 0:D],
                                     lhsT=xT16[:, PART * c:PART * (c + 1)],
                                     rhs=wv16[:], start=True, stop=True)
                    nc.vector.tensor_copy(vp[:, c, 0:D], pv[:, 0:D])
                    # Q row-tile [128, 64] -> DRAM (padded row stride 128)
                    pq32 = ps_qr.tile([PART, D], f32, tag="qr32")
                    nc.tensor.matmul(out=pq32[:],
                                     lhsT=xT16[:, PART * c:PART * (c + 1)],
                                     rhs=wq16[:], start=True, stop=True)
                    pq = mn.tile([PART, D], bf16, tag="qr_st")
                    nc.vector.tensor_copy(pq[:], pq32[:])
                    nc.sync.dma_start(out=q_rows_re[:, c, 0:D], in_=pq[:])

            bstats = sc.tile([D, 8, 6], f32, tag="bstats")
            wqT = sc.tile([D, D], f32, tag="wqT")

            load_tiles(0, 4)
            pwt = ps_pj.tile([D, 512], f32, tag="pj")
            nc.tensor.transpose(out=pwt[:, 0:D], in_=wq_s[:], identity=ident[0:D, 0:D])
            nc.vector.tensor_copy(wqT[:], pwt[:, 0:D])
            load_tiles(4, 8)
            load_tiles(8, 12)
            for s in range(8):
                proj_slab(s)
                if s < 5:
                    load_tiles(4 * s + 12, 4 * s + 16)
                proj_qv(4 * s, 4 * s + 4)
            nc.vector.memset(vp[:, :, D:D + 1], 1.0)

        # ---- K_reduce (DVE; channel = partition of kT32) ----
        aggr = sc.tile([D, 2], f32, tag="aggr")
        nc.vector.bn_aggr(aggr[:], bstats[:])
        sig = sc.tile([D, 1], f32, tag="sig")
        nc.vector.memset(sig[:], 1.0)
        for _ in range(2):
            rs = sc.tile([D, 1], f32, tag="rs")
            nc.vector.reciprocal(rs[:], sig[:])
            nc.vector.tensor_tensor(out=rs[:], in0=rs[:], in1=aggr[:, 1:2], op=OP.mult)
            nc.vector.tensor_tensor(out=rs[:], in0=rs[:], in1=sig[:], op=OP.add)
            nc.vector.tensor_scalar_mul(sig[:], rs[:], 0.5)
        tk = sc.tile([D, 1], f32, tag="tk")
        nc.vector.tensor_scalar(out=tk[:], in0=sig[:], scalar1=float(Z),
                                scalar2=None, op0=OP.mult)
        nc.vector.tensor_tensor(out=tk[:], in0=tk[:], in1=aggr[:, 0:1], op=OP.add)
        cnt_c = sc.tile([D, 1], f32, tag="cnt_c")
        nc.vector.tensor_scalar(out=tmp1m[:], in0=kT32[:], scalar1=tk[:, 0:1],
                                scalar2=None, op0=OP.is_gt, op1=OP.add,
                                accum_out=cnt_c[:])
        adj = sc.tile([D, 1], f32, tag="adj")
        nc.vector.tensor_scalar(out=adj[:], in0=cnt_c[:], scalar1=float(-LQ),
                                scalar2=1.0 / (L * PHI), op0=OP.add, op1=OP.mult)
        nc.vector.tensor_tensor(out=adj[:], in0=adj[:], in1=sig[:], op=OP.mult)
        t1 = sc.tile([D, 1], f32, tag="t1")
        nc.vector.tensor_tensor(out=t1[:], in0=tk[:], in1=adj[:], op=OP.add)
        s1c = sc.tile([D, 1], f32, tag="s1c")
        nc.vector.tensor_scalar(out=tmp1m[:], in0=kT32[:], scalar1=t1[:, 0:1],
                                scalar2=0.0, op0=OP.subtract, op1=OP.max)
        nc.vector.tensor_scalar(out=tmp1m[:], in0=tmp1m[:], scalar1=0.0,
                                scalar2=None, op0=OP.add, op1=OP.add,
                                accum_out=s1c[:])
        nc.vector.tensor_scalar(out=kr[:], in0=s1c[:], scalar1=1.0 / LQ,
                                scalar2=None, op0=OP.mult)
        nc.vector.tensor_tensor(out=kr[:], in0=kr[:], in1=t1[:], op=OP.add)

        # =============== phase 2: selection then selected attention ===============
        with tc.tile_pool(name="ps_strip", bufs=3, space="PSUM") as ps_strip, \
             tc.tile_pool(name="ps_acc", bufs=1, space="PSUM") as ps_acc, \
             tc.tile_pool(name="ps_mis", bufs=1, space="PSUM") as ps_mis:

            # ---- w = Wq @ Kr ; sqk (f32, exact) ----
            pw = ps_mis.tile([PART, 512], f32, tag="mis")
            nc.tensor.matmul(out=pw[0:D, 0:1], lhsT=wqT[:], rhs=kr[:],
                             start=True, stop=True)
            nc.vector.tensor_copy(wvec[:], pw[0:D, 0:1])
            psq = ps_mis.tile([PART, 512], f32, tag="mis")
            for c in range(NT):
                nc.tensor.matmul(out=psq[:, c:c + 1],
                                 lhsT=xT32[:, PART * c:PART * (c + 1)],
                                 rhs=wvec[:], start=True, stop=True)
            nc.vector.tensor_copy(sqk[:], psq[:, 0:NT])

            # replicate sqk into every partition via DRAM round-trip
            psqT = ps_mis.tile([PART, 512], f32, tag="mis")
            nc.tensor.transpose(out=psqT[0:NT, 0:PART], in_=sqk[:], identity=ident[:])
            sqkT = sc.tile([NT, PART], f32, tag="sqkT")
            nc.vector.tensor_copy(sqkT[:], psqT[0:NT, 0:PART])
            nc.sync.dma_start(out=scr_row[:], in_=sqkT[:])
            for h in range(2):
                hs = slice(2048 * h, 2048 * (h + 1))
                nc.sync.dma_start(out=sqk_rep[:, hs],
                                  in_=scr_row[:, hs].to_broadcast([NCAND, 2048]))

            if debug:
                nc.sync.dma_start(out=dbg["dbg_kr"][:], in_=kr[:])
                nc.sync.dma_start(out=dbg["dbg_sqk"][:], in_=sqk[:])

            # ---- meanV (PE accumulate) while the broadcast is in flight ----
            pmv = ps_mis.tile([PART, 512], f32, tag="mis")
            for c in range(NT):
                nc.tensor.matmul(out=pmv[0:D + 1, 0:1], lhsT=vp[:, c, :], rhs=onesb[:],
                                 start=(c == 0), stop=(c == NT - 1))
            mv_col = sc.tile([D, 1], f32, tag="mv_col")
            nc.vector.tensor_scalar_mul(mv_col[:], pmv[0:D, 0:1], 1.0 / L)
            pmvT = ps_mis.tile([PART, 512], f32, tag="mis")
            nc.tensor.transpose(out=pmvT[0:1, 0:D], in_=mv_col[:],
                                identity=ident[0:D, 0:D])
            mv_row = sc.tile([1, D], f32, tag="mv_row")
            nc.vector.tensor_copy(mv_row[:], pmvT[0:1, 0:D])
            pmvF = ps_mis.tile([PART, 512], f32, tag="mis")
            nc.tensor.matmul(out=pmvF[:, 0:D], lhsT=ones1x128[:], rhs=mv_row[:],
                             start=True, stop=True)
            nc.vector.tensor_copy(mvf[:], pmvF[:, 0:D])

            # ---- 4-pass 65-ary threshold search (exact top-LQ) ----
            lo = mn.tile([NCAND, 1], f32, tag="lo_a")
            nc.vector.memset(lo[:], -BOUND)
            dlt = mn.tile([NCAND, 1], f32, tag="dlt_a")
            nc.vector.memset(dlt[:], 2.0 * BOUND / 65.0)
            for it in range(N_PASS):
                tvec = mn.tile([NCAND, 1], f32, tag=f"tv{it % 2}")
                nc.vector.tensor_tensor(out=tvec[:], in0=pidx1[0:NCAND, :], in1=dlt[:], op=OP.mult)
                nc.vector.tensor_tensor(out=tvec[:], in0=tvec[:], in1=lo[:], op=OP.add)
                cntq = mn.tile([NCAND, 1], f32, tag="cntq")
                nc.vector.tensor_scalar(out=cmp_rep[0:NCAND, :], in0=sqk_rep[:],
                                        scalar1=tvec[:, 0:1], scalar2=None,
                                        op0=OP.is_gt, op1=OP.add, accum_out=cntq[:])
                sel = mn.tile([NCAND, 1], f32, tag="sel")
                nc.vector.tensor_scalar(out=sel[:], in0=cntq[:], scalar1=float(LQ),
                                        scalar2=None, op0=OP.is_ge)
                jsr = mn.tile([NCAND, 1], f32, tag="jsr")
                nc.gpsimd.partition_all_reduce(jsr[:], sel[:], channels=NCAND,
                                               reduce_op=bass_isa.ReduceOp.add)
                step = mn.tile([NCAND, 1], f32, tag="step")
                nc.vector.tensor_tensor(out=step[:], in0=jsr[:], in1=dlt[:], op=OP.mult)
                nlo = mn.tile([NCAND, 1], f32, tag=f"lo_{'b' if it % 2 == 0 else 'a'}")
                nc.vector.tensor_tensor(out=nlo[:], in0=lo[:], in1=step[:], op=OP.add)
                ndl = mn.tile([NCAND, 1], f32, tag=f"dlt_{'b' if it % 2 == 0 else 'a'}")
                nc.vector.tensor_scalar_mul(ndl[:], dlt[:], 1.0 / 65.0)
                lo, dlt = nlo, ndl
            lo128 = mn.tile([PART, 1], f32, tag="lo128")
            nc.gpsimd.partition_broadcast(lo128[:], lo[0:1, :], channels=PART)
            nc.vector.tensor_scalar(out=mask[:], in0=sqk[:], scalar1=lo128[:, 0:1],
                                    scalar2=None, op0=OP.is_gt)

            # ---- compact selected indices on GPSIMD ----
            # sel_or_neg[p, c] = (q+1)*mask - 1  (q = c*128 + p)
            son = sc.tile([PART, NT], f32, tag="son")
            nc.vector.tensor_tensor(out=son[:], in0=qio[:], in1=mask[:], op=OP.mult)
            nc.vector.tensor_scalar(out=son[:], in0=son[:], scalar1=-1.0,
                                    scalar2=None, op0=OP.add)
            # reshape [128, 32] -> [16, 256] via DRAM (order irrelevant)
            nc.sync.dma_start(out=scr_sel[:], in_=son[:])
            wrap = sc.tile([16, 256], f32, tag="wrap")
            nc.sync.dma_start(
                out=wrap[:].rearrange("r (g c) -> r g c", g=8),
                in_=scr_sel[:].rearrange("(g r) c -> r g c", r=16))
            sg_out = sc.tile([16, NSEL // 16], f32, tag="sg_out")
            n_found = sc.tile([1, 1], u32, tag="n_found")
            nc.gpsimd.sparse_gather(sg_out[:], wrap[:], num_found=n_found[:])
            # tail mask: entry j (at [r, f], j = 16f + r) valid iff j < count
            nff = sc.tile([1, 1], f32, tag="nff")
            nc.vector.tensor_copy(nff[:], n_found[:])
            nfb = sc.tile([16, 1], f32, tag="nfb")
            nc.gpsimd.partition_broadcast(nfb[:], nff[:], channels=16)
            thr = sc.tile([16, 1], f32, tag="thr")
            nc.vector.tensor_tensor(out=thr[:], in0=nfb[:], in1=rio[:], op=OP.subtract)
            nc.vector.tensor_scalar(out=thr[:], in0=thr[:], scalar1=15.0,
                                    scalar2=0.0625, op0=OP.add, op1=OP.mult)
            thr_i = sc.tile([16, 1], i16, tag="thr_i")
            nc.vector.tensor_copy(thr_i[:], thr[:])     # trunc -> floor
            thr_f = sc.tile([16, 1], f32, tag="thr_f")
            nc.vector.tensor_copy(thr_f[:], thr_i[:])
            vmask = sc.tile([16, NSEL // 16], f32, tag="vmask")
            nc.vector.tensor_scalar(out=vmask[:], in0=fio[:], scalar1=thr_f[:, 0:1],
                                    scalar2=None, op0=OP.is_lt)
            idxf = sc.tile([16, NSEL // 16], f32, tag="idxf")
            nc.vector.tensor_scalar(out=idxf[:], in0=sg_out[:], scalar1=1.0,
                                    scalar2=None, op0=OP.add)
            nc.vector.tensor_tensor(out=idxf[:], in0=idxf[:], in1=vmask[:], op=OP.mult)
            nc.vector.tensor_scalar(out=idxf[:], in0=idxf[:], scalar1=-1.0,
                                    scalar2=None, op0=OP.add)
            nc.vector.tensor_copy(idxbo[:, 0, :], idxf[:])
            nc.vector.tensor_scalar_max(idxbo[:, 1, :], idxbo[:, 0, :], 0.0)
            # replicate both idx variants to 128 partitions (8 SBUF->SBUF DMAs)
            for g in range(8):
                nc.sync.dma_start(
                    out=idx2[16 * g:16 * (g + 1), :, :].rearrange("r t f -> r (t f)"),
                    in_=idxbo[:].rearrange("r t f -> r (t f)"))
            if debug:
                dbg_i = sc.tile([16, NSEL // 16], f32, tag="dbg_i")
                nc.vector.tensor_copy(dbg_i[:], idxbo[:, 0, :])
                nc.sync.dma_start(out=dbg["dbg_idx"][:], in_=dbg_i[:])
                nc.sync.dma_start(out=dbg["dbg_cnt"][:], in_=nff[:])
                nc.sync.dma_start(out=dbg["dbg_mask"][:], in_=mask[:])

            # ---- gather selected Q columns per slab (pipelined) ----
            col0 = [0]
            for w_ in SLABS:
                col0.append(col0[-1] + w_)
            for s, w_ in enumerate(SLABS):
                nc.gpsimd.dma_gather(
                    out_ap=qselT[:, col0[s]:col0[s] + w_].rearrange(
                        "p (o n) -> p o n", o=1),
                    in_ap=q_rows[:],
                    idxs_ap=idx2[:, 1, col0[s] // 16:(col0[s] + w_) // 16],
                    num_idxs=w_,
                    num_idxs_reg=w_,
                    elem_size=PART,
                    transpose=True,
                )

            # ---- base fill: res = meanV * (1 - mask); zero where selected ----
            invm = sc.tile([PART, NT], f32, tag="invm")
            nc.vector.tensor_scalar(out=invm[:], in0=mask[:], scalar1=1.0,
                                    scalar2=-1.0, op0=OP.subtract, op1=OP.mult)
            for c in range(NT):
                rb = mn.tile([PART, D], f32, tag="res_b")
                nc.gpsimd.tensor_scalar(out=rb[:], in0=mvf[:],
                                        scalar1=invm[:, c:c + 1], scalar2=None,
                                        op0=OP.mult)
                nc.sync.dma_start(out=out_re[:, c, :], in_=rb[:])

            # ---- selected attention: slabs over the gathered columns ----
            cnt_reg = nc.gpsimd.alloc_register("cnt_sc")
            nc.gpsimd.reg_load(cnt_reg, n_found[:])

            def pt_of(s):
                return pt_a if s % 2 == 0 else pt_b

            def emit_scores(s):
                w = SLABS[s]
                c0 = col0[s]
                ptc = pt_of(s)
                for gi, (g0, glen) in enumerate(GROUPS):
                    strip = ps_strip.tile([PART, 2, 512], f32, tag="strip")
                    for i in range(glen):
                        j = g0 + i
                        nc.tensor.matmul(out=strip[:, i, 0:w],
                                         lhsT=kT[:, PART * j:PART * (j + 1)],
                                         rhs=qselT[0:D, c0:c0 + w],
                                         start=True, stop=True)
                    emit_exp(gi, strip[:, 0:glen, 0:w], ptc, g0, glen, w)

            def emit_av(s):
                w = SLABS[s]
                ptp = pt_of(s)
                oT = ps_acc.tile([D + 1, 512], f32, tag="oT")
                for j in range(NT):
                    nc.tensor.matmul(out=oT[:, 0:w], lhsT=vp[:, j, :],
                                     rhs=ptp[:, j, 0:w],
                                     start=(j == 0), stop=(j == NT - 1))
                oT_sb = mn.tile([D + 1, 512], f32, tag="oT_sb")
                nc.vector.tensor_copy(oT_sb[:, 0:w], oT[:, 0:w])
                return oT_sb

            def emit_norm(s, oT_sb):
                w = SLABS[s]
                for i in range(w // PART):
                    ct = col0[s] // PART + i
                    po = ps_mis.tile([PART, 512], f32, tag="mis")
                    nc.tensor.transpose(out=po[:, 0:D + 1],
                                        in_=oT_sb[:, PART * i:PART * (i + 1)],
                                        identity=ident[0:D + 1, 0:D + 1])
                    dcol = mn.tile([PART, 1], f32, tag="dcol")
                    nc.vector.tensor_copy(dcol[:], po[:, D:D + 1])
                    rec = mn.tile([PART, 1], f32, tag="rec")
                    nc.vector.reciprocal_approx_fast(rec[:], dcol[:])
                    nc.vector.tensor_scalar(out=res_sel[:, ct, :], in0=po[:, 0:D],
                                            scalar1=rec[:, 0:1], scalar2=None,
                                            op0=OP.mult)
                # scatter this slab's normalized rows into out (add onto zeros)
                sreg = nc.gpsimd.alloc_register(f"cnt_s{s}")
                nc.gpsimd.reg_alu(sreg, cnt_reg, col0[s], OP.subtract)
                nc.gpsimd.reg_alu(sreg, sreg, 0, OP.max)
                nc.gpsimd.reg_alu(sreg, sreg, w, OP.min)
                nc.gpsimd.dma_scatter_add(
                    out_ap=out[:],
                    in_ap=res_sel[:, 4 * s:4 * s + w // PART, :],
                    idxs_ap=idx2[:, 0, col0[s] // 16:(col0[s] + w) // 16],
                    num_idxs=w,
                    num_idxs_reg=sreg,
                    elem_size=D,
                )

            done = {}
            for s in range(len(SLABS) + 1):
                if s < len(SLABS):
                    emit_scores(s)
                if s >= 1:
                    done[s - 1] = emit_av(s - 1)
                if s >= 2:
                    emit_norm(s - 2, done.pop(s - 2))
            emit_norm(len(SLABS) - 1, done.pop(len(SLABS) - 1))

    nc.finalize()
    return nc


_CACHE = {}


def _get_nc(debug=False):
    key = bool(debug)
    if key not in _CACHE:
        _CACHE[key] = build(debug=key)
    return _CACHE[key]


def kernel(x, Wq, Wk, Wv, debug=False):
    nc = _get_nc(debug=debug)
    x = np.asarray(x, dtype=np.float32)
    in_maps = [
        {"x": np.ascontiguousarray(x[i]),
         "Wq": np.asarray(Wq, np.float32), "Wk": np.asarray(Wk, np.float32),
         "Wv": np.asarray(Wv, np.float32)}
        for i in range(B)
    ]
    last_err = None
    for _attempt in range(3):
        try:
            r = run_bass_kernel_spmd(nc, in_maps, core_ids=list(range(N_CORES)))
            out = np.stack([r.results[i]["out"] for i in range(B)]).astype(np.float32)
            break
        except Exception as e:  # transient axon RPC failures
            last_err = e
    else:
        raise last_err
    if debug:
        return out, r.results
    return out
